# revision 22
# baseline (speedup 1.0000x reference)
"""Multi-Latent Attention TRN2 kernel, v2.

Sharding: hybrid batch x heads. 8 cores = 2 batches x 4 head-groups.
Each core handles ONE batch and 4 of the 16 heads; the host sums the 4
partial output projections per batch and adds a constant bias vector.

Algebraic restructuring vs the straightforward dataflow (all exact):
  * Wkr is folded into the query projection on the host:
      scores = q . k = (X Wq_h Wkr^T) . latk   =>  W'_h = Wq_h @ Wkr^T
    so the key reconstruction matmul disappears and the query projection
    halves (contract L=64 latents per head). Key-side biases (blk, bkr)
    shift every score of a given query equally -> cancel in softmax.
  * Value latents are produced TOKEN-major (lhsT = X^T tile, no extra
    transpose); even heads carry a ones column so their U' matmul
      U'aug = [Lv | 1]^T P~^T
    yields the softmax denominator as a free 65th output row. (Odd
    heads sit at PSUM base 64; a 65-row output would start at the
    illegal base 63, so they get an explicit 1-row ones-column chain
    into row 0 of their own PSUM tile instead.)
  * Wvr is folded into the output projection on the host:
      attn Wvr Wo_h = (P~ Lv / denom) (Wvr Wo_h)  =>  W''_h = Wvr @ Wo_h
    so the value reconstruction matmul disappears and the final
    projection contracts 64 latents per head, two heads stacked = 128.
  * blv/bvr/bo contribute a constant vector per output row (softmax
    rows sum to 1); the host adds const = sum_h(blv_h@Wvr+bvr)@Wo_h+bo.

Head pairs are stacked on SBUF partitions (even head -> partitions
0-63, odd head -> 64-127) so the projections and the final matmul use
the full 128-row PE array.
"""

import math
from contextlib import ExitStack

import numpy as np

import concourse.mybir as mybir
from concourse import bacc
from concourse.bass import ds, ts
from concourse.tile import TileContext

# Problem constants (hardcoded per contract).
B, S, D = 2, 2048, 2048
H, DK, DV, L = 16, 128, 128, 64
N_CORES = 8
BPC = N_CORES // B        # cores per batch = 4
HPC = H // BPC            # heads per core = 4
NP = HPC // 2             # head pairs per core = 2
SB = S                    # tokens per core (one batch) = 2048
KO = D // 128             # contraction k-tiles over D = 16
NTT = SB // 128           # 128-token tiles = 16
NQB = SB // 512           # 512-wide q blocks = 4
CHUNK = 512               # projection token chunk
NCH = SB // CHUNK         # = 4
TPC = CHUNK // 128        # token tiles per chunk = 4
LVW = HPC * (L + 1)       # Lv_tok free width = 260

F32 = mybir.dt.float32
BF16 = mybir.dt.bfloat16

INV_SQRT_DK = 1.0 / math.sqrt(DK)
EXPF = mybir.ActivationFunctionType.Exp


def build_kernel():
    nc = bacc.Bacc(trn_type="TRN2", debug=False, num_swdge_queues=4)

    # ---- DRAM I/O (per-core shards, host-prepped) ----
    # X^T pre-tiled on the host: [NCH*128, KO*CHUNK], so each partition
    # reads one contiguous 16KB run per chunk (1KB packets starve the DMA)
    qT = nc.dram_tensor("qT", [NCH * 128, KO * CHUNK], BF16,
                        kind="ExternalInput")
    kT = nc.dram_tensor("kT", [NCH * 128, KO * CHUNK], BF16,
                        kind="ExternalInput")
    vT = nc.dram_tensor("vT", [NCH * 128, KO * CHUNK], BF16,
                        kind="ExternalInput")
    # W' = Wq_h @ Wkr^T, 4 heads, pre-swizzled [128, KO*256]
    wqp = nc.dram_tensor("wqp", [128, KO * 256], BF16, kind="ExternalInput")
    bqp = nc.dram_tensor("bqp", [128, NP], F32, kind="ExternalInput")
    wlk = nc.dram_tensor("wlk", [128, KO * 256], BF16, kind="ExternalInput")
    wlv = nc.dram_tensor("wlv", [128, KO * 256], BF16, kind="ExternalInput")
    # W'' = Wvr @ Wo_h, head-pair stacked: [128, NP*D]
    wop = nc.dram_tensor("wop", [128, NP * D], BF16, kind="ExternalInput")
    outp = nc.dram_tensor("outp", [SB, D], BF16, kind="ExternalOutput")
    import os
    dbg = bool(os.environ.get("MLA_DEBUG"))
    if dbg:
        d_qsb = nc.dram_tensor("d_qsb", [128, NP * SB], BF16,
                               kind="ExternalOutput")
        d_ksb = nc.dram_tensor("d_ksb", [128, NP * SB], BF16,
                               kind="ExternalOutput")
        d_vsb = nc.dram_tensor("d_vsb", [128, NTT * LVW], BF16,
                               kind="ExternalOutput")
        d_asb = nc.dram_tensor("d_asb", [128, NP * SB], BF16,
                               kind="ExternalOutput")
        d_xq2 = nc.dram_tensor("d_xq2", [128, KO * CHUNK], BF16,
                               kind="ExternalOutput")

    with TileContext(nc) as tc, ExitStack() as ctx:
        ec = ctx.enter_context
        consts = ec(tc.tile_pool(name="consts", bufs=1))
        persist = ec(tc.tile_pool(name="persist", bufs=1))
        xqp = ec(tc.tile_pool(name="xqp", bufs=2))
        xkp = ec(tc.tile_pool(name="xkp", bufs=2))
        xvp = ec(tc.tile_pool(name="xvp", bufs=2))
        ptpool = ec(tc.tile_pool(name="ptpool", bufs=2))
        statpool = ec(tc.tile_pool(name="statpool", bufs=1))
        opool = ec(tc.tile_pool(name="opool", bufs=1))
        psp = ec(tc.tile_pool(name="psp", bufs=2, space="PSUM"))
        pss = ec(tc.tile_pool(name="pss", bufs=2, space="PSUM"))
        psu = ec(tc.tile_pool(name="psu", bufs=2, space="PSUM"))
        psf = ec(tc.tile_pool(name="psf", bufs=2, space="PSUM"))

        # ---- weights / constants / first-chunk inputs ----
        wq_sb = consts.tile([128, KO, 256], BF16, tag="wqp")
        nc.sync.dma_start(wq_sb, wqp.rearrange("p (ko m) -> p ko m", ko=KO))

        def xsl(x, c):
            return x[ds(c * 128, 128), :].rearrange(
                "p (ko t) -> p ko t", ko=KO)

        xq0 = xqp.tile([128, KO, CHUNK], BF16, tag="xq")
        nc.sync.dma_start(xq0, xsl(qT, 0))
        wlk_sb = consts.tile([128, KO, 256], BF16, tag="wlk")
        nc.scalar.dma_start(wlk_sb, wlk.rearrange("p (ko m) -> p ko m", ko=KO))
        xk0 = xkp.tile([128, KO, CHUNK], BF16, tag="xk")
        nc.scalar.dma_start(xk0, xsl(kT, 0))
        wlv_sb = consts.tile([128, KO, 256], BF16, tag="wlv")
        nc.gpsimd.dma_start(wlv_sb, wlv.rearrange("p (ko m) -> p ko m", ko=KO))
        xv0 = xvp.tile([128, KO, CHUNK], BF16, tag="xv")
        nc.gpsimd.dma_start(xv0, xsl(vT, 0))
        wo_sb = consts.tile([128, NP, D], BF16, tag="wop")
        nc.gpsimd.dma_start(wo_sb, wop.rearrange("p (n d) -> p n d", n=NP))
        bq_sb = consts.tile([128, NP], F32, tag="bqp")
        nc.scalar.dma_start(bq_sb, bqp[:, :])

        # causal mask for a diagonal 128x128 block of P~^T: 1 where k <= q
        maskT = consts.tile([128, 128], BF16, tag="maskT")
        nc.gpsimd.memset(maskT, 1.0)
        nc.gpsimd.affine_select(
            out=maskT, in_=maskT, compare_op=mybir.AluOpType.is_ge,
            fill=0.0, base=0, pattern=[[1, 128]], channel_multiplier=-1,
        )
        ones_bf = consts.tile([128, 64], BF16, tag="ones_bf")
        nc.gpsimd.memset(ones_bf, 1.0)

        # persistent activations
        qsb = persist.tile([128, NP, SB], BF16, tag="qsb")
        ksb = persist.tile([128, NP, SB], BF16, tag="ksb")
        vsb = persist.tile([128, NTT, LVW], BF16, tag="vsb")
        asb = persist.tile([128, NP, SB], BF16, tag="asb")

        # ones columns of vsb: even head h: col h*65+64; odd head: col h*65
        nc.gpsimd.memset(vsb[:, :, ds(L, 2)], 1.0)          # cols 64,65
        nc.gpsimd.memset(vsb[:, :, ds(2 * 65 + L, 2)], 1.0)  # cols 194,195

        def proj_chunk(c, xq, xk, xv):
            """Projections for token chunk c (CHUNK=512 tokens)."""
            # q' and latk: feature-major, pair-stacked partitions
            for (w_sb, xin, dst, bias) in (
                (wq_sb, xq, qsb, True), (wlk_sb, xk, ksb, False)
            ):
                for p in range(NP):
                    psq = psp.tile([128, 512], F32, tag="pp")
                    for ko in range(KO):
                        nc.tensor.matmul(
                            psq,
                            w_sb[:, ko, ts(p, 128)],
                            xin[:, ko, :],
                            start=(ko == 0), stop=(ko == KO - 1),
                        )
                    dsl = dst[:, p, ds(c * CHUNK, CHUNK)]
                    if bias:
                        nc.vector.tensor_scalar_add(dsl, psq,
                                                    bq_sb[:, p : p + 1])
                    else:
                        nc.any.tensor_copy(out=dsl, in_=psq)
            # value latents, token-major (lhsT = X^T tile); 2 token tiles
            # packed per PSUM bank
            for t2 in range(TPC // 2):
                psv = psp.tile([128, 512], F32, tag="pp")
                for tl in range(2):
                    for ko in range(KO):
                        nc.tensor.matmul(
                            psv[:, ts(tl, 256)],
                            xv[:, ko, ds((t2 * 2 + tl) * 128, 128)],
                            wlv_sb[:, ko, :],
                            start=(ko == 0), stop=(ko == KO - 1),
                        )
                for tl in range(2):
                    tt = c * TPC + t2 * 2 + tl
                    for h in range(HPC):
                        # even head latents -> cols h*65..h*65+63,
                        # odd head latents -> cols h*65+1..h*65+64
                        dcol = h * 65 + (h % 2)
                        nc.any.tensor_copy(
                            out=vsb[:, tt, ds(dcol, L)],
                            in_=psv[:, ds(tl * 256 + h * L, L)],
                        )

        def attn_head(Q, p, o):
            """Attention for q-block Q (512 queries), head pair p, member o."""
            h = 2 * p + o
            jmax = 4 * Q + 4
            po = 64 * o  # partition offset of this head in pair-stacked tiles
            ptq = ptpool.tile([128, NTT, 512], BF16, tag="pt")

            # U' accumulation interleaved two behind scores so the PE is
            # never stalled on the exp. Even head: lhsT = [latents | ones]
            # -> rows 0-63 latents + row 64 denominator. Odd head:
            # latents-only lhsT at out base 64 -> rows 64-127; denominator
            # via a second 1-row chain into row 0 of the same tile.
            ps_u = psu.tile([128, 512], F32, tag="u")
            lcol = h * 65 if o == 0 else h * 65 + 1
            lw = 65 if o == 0 else 64

            def u_step(j):
                qoff = max(0, (j - 4 * Q) * 128)
                nc.tensor.matmul(
                    ps_u[ds(po, lw), qoff:],
                    vsb[:, j, ds(lcol, lw)],
                    ptq[:, j, qoff:],
                    start=(j == 0), stop=(j == jmax - 1),
                    skip_group_check=True,
                )
                if o == 1:
                    nc.tensor.matmul(
                        ps_u[ds(0, 1), qoff:],
                        vsb[:, j, ds(h * 65, 1)],
                        ptq[:, j, qoff:],
                        start=(j == 0), stop=(j == jmax - 1),
                        skip_group_check=True,
                    )

            for j in range(jmax):
                qoff = max(0, (j - 4 * Q) * 128)
                n = 512 - qoff
                ps_s = pss.tile([128, 512], F32, tag="st")
                nc.tensor.matmul(
                    ps_s[:, :n],
                    ksb[ds(po, 64), p, ts(j, 128)],
                    qsb[ds(po, 64), p, ds(Q * 512 + qoff, n)],
                    start=True, stop=True,
                )
                nc.scalar.activation(
                    ptq[:, j, ds(qoff, n)], ps_s[:, :n], EXPF,
                    scale=INV_SQRT_DK,
                )
                if j >= 4 * Q:  # diagonal k-tile: causal mask
                    nc.vector.tensor_tensor(
                        ptq[:, j, ds(qoff, 128)],
                        ptq[:, j, ds(qoff, 128)],
                        maskT, mybir.AluOpType.mult,
                    )
                if j >= 2:
                    u_step(j - 2)
            u_step(jmax - 2)
            u_step(jmax - 1)

            a_sl = asb[ds(po, 64), p, ds(Q * 512, 512)]
            if o == 0:
                # cast denominator (row 64) to bf16, PE-broadcast it to
                # rows 0-63, then take the reciprocal at base 0 (custom
                # DVE ops silently break at non-zero partition offsets)
                den_b = statpool.tile([128, 512], BF16, tag="rcb")
                nc.vector.tensor_copy(out=den_b[ds(64, 1), :],
                                      in_=ps_u[ds(64, 1), :])
                denb_ps = psf.tile([128, 512], F32, tag="f")
                nc.tensor.matmul(
                    denb_ps[ds(0, 64), :],
                    ones_bf[ds(64, 1), :],
                    den_b[ds(64, 1), :],
                    start=True, stop=True,
                )
                rcpb = statpool.tile([128, 512], F32, tag="rcb3")
                nc.vector.reciprocal_approx_fast(
                    out=rcpb[ds(0, 64), :], in_=denb_ps[ds(0, 64), :]
                )
                nc.vector.tensor_tensor(
                    a_sl, ps_u[ds(0, 64), :], rcpb[ds(0, 64), :],
                    mybir.AluOpType.mult,
                )
            else:
                # reciprocal of the denominator chain (row 0, base 0 where
                # custom DVE ops are safe), cast bf16, PE-broadcast to
                # rows 64-127, hop through SBUF for the two-operand TT
                rcp_f = statpool.tile([128, 512], F32, tag="rcf")
                nc.vector.reciprocal_approx_fast(
                    out=rcp_f[ds(0, 1), :], in_=ps_u[ds(0, 1), :]
                )
                rcp_b = statpool.tile([128, 512], BF16, tag="rcb2")
                nc.vector.tensor_copy(out=rcp_b[ds(0, 1), :],
                                      in_=rcp_f[ds(0, 1), :])
                rcpb_ps = psf.tile([128, 512], F32, tag="f")
                nc.tensor.matmul(
                    rcpb_ps[ds(64, 64), :],
                    ones_bf[ds(0, 1), :],
                    rcp_b[ds(0, 1), :],
                    start=True, stop=True,
                )
                rcpb = statpool.tile([128, 512], F32, tag="rcb3")
                nc.vector.tensor_copy(out=rcpb[ds(64, 64), :],
                                      in_=rcpb_ps[ds(64, 64), :])
                nc.vector.tensor_tensor(
                    a_sl, ps_u[ds(64, 64), :], rcpb[ds(64, 64), :],
                    mybir.AluOpType.mult,
                )

        def final_block(Q, tiles=range(4)):
            """Output projection for q-block Q's token tiles."""
            for tl in tiles:
                tt = Q * 4 + tl
                o_sb = opool.tile([128, D], BF16, tag="o")
                for dc in range(D // 512):
                    ps_f = psf.tile([128, 512], F32, tag="f")
                    for p in range(NP):
                        nc.tensor.matmul(
                            ps_f,
                            asb[:, p, ts(tt, 128)],
                            wo_sb[:, p, ts(dc, 512)],
                            start=(p == 0), stop=(p == NP - 1),
                        )
                    nc.any.tensor_copy(out=o_sb[:, ts(dc, 512)], in_=ps_f)
                nc.sync.dma_start(outp[ts(tt, 128), :], o_sb)

        # ---- schedule ----
        # q-block Q needs only chunks 0..Q. Chunk-0 DMA gets exclusive
        # bandwidth (prefetches are demoted into the attention blocks),
        # each proj sits just before its consumer, and final(2) splits
        # per token tile to fill attn(3)'s inter-head normalize stalls.
        def prefetch(c):
            xq = xqp.tile([128, KO, CHUNK], BF16, tag="xq")
            nc.sync.dma_start(xq, xsl(qT, c))
            xk = xkp.tile([128, KO, CHUNK], BF16, tag="xk")
            nc.scalar.dma_start(xk, xsl(kT, c))
            xv = xvp.tile([128, KO, CHUNK], BF16, tag="xv")
            nc.gpsimd.dma_start(xv, xsl(vT, c))
            return (xq, xk, xv)

        xs = {0: (xq0, xk0, xv0)}
        proj_chunk(0, *xs[0])
        for Q in range(3):
            attn_head(Q, 0, 0)
            if Q + 1 < NCH:
                xs[Q + 1] = prefetch(Q + 1)
            attn_head(Q, 0, 1)
            attn_head(Q, 1, 0)
            attn_head(Q, 1, 1)
            final_block(Q, range(2) if Q == 2 else range(4))
            proj_chunk(Q + 1, *xs[Q + 1])
        attn_head(3, 0, 0)
        final_block(2, [2])
        attn_head(3, 0, 1)
        final_block(2, [3])
        attn_head(3, 1, 0)
        attn_head(3, 1, 1)
        final_block(3)

        if dbg:
            nc.sync.dma_start(
                d_qsb.rearrange("p (n t) -> p n t", n=NP), qsb)
            nc.sync.dma_start(
                d_ksb.rearrange("p (n t) -> p n t", n=NP), ksb)
            nc.sync.dma_start(
                d_vsb.rearrange("p (n t) -> p n t", n=NTT), vsb)
            nc.sync.dma_start(
                d_asb.rearrange("p (n t) -> p n t", n=NP), asb)
            nc.sync.dma_start(
                d_xq2.rearrange("p (ko t) -> p ko t", ko=KO), xs[2][0])

    nc.finalize()
    return nc


_NC_CACHE = None


def _get_nc():
    global _NC_CACHE
    if _NC_CACHE is None:
        _NC_CACHE = build_kernel()
    return _NC_CACHE


def _prep_in_maps(queries, keys, values, Wq, bq, Wlk, blk, Wlv, blv,
                  Wkr, bkr, Wvr, bvr, Wo, bo):
    import ml_dtypes

    f = np.float32
    bf = ml_dtypes.bfloat16

    def swz(w):
        """[D, 256] -> [128, KO*256] so each partition reads contiguously."""
        return np.ascontiguousarray(
            w.reshape(KO, 128, 256).transpose(1, 0, 2).reshape(128, KO * 256)
            .astype(bf)
        )

    Wq64 = np.asarray(Wq, np.float64).reshape(D, H, DK)
    Wkr64 = np.asarray(Wkr, np.float64)
    Wvr64 = np.asarray(Wvr, np.float64)
    Wo64 = np.asarray(Wo, np.float64).reshape(H, DV, D)
    bq64 = np.asarray(bq, np.float64).reshape(H, DK)

    # W'_h = Wq_h @ Wkr^T : [D, L] per head;  b'_h = bq_h @ Wkr^T
    Wqp = np.einsum("dhk,lk->dhl", Wq64, Wkr64)          # [D, H, L]
    bqp_all = np.einsum("hk,lk->hl", bq64, Wkr64)        # [H, L]
    # W''_h = Wvr @ Wo_h : [L, D] per head
    Wop = np.einsum("lv,hvd->hld", Wvr64, Wo64)          # [H, L, D]

    # constant output vector: biases through softmax-invariant paths
    blv64 = np.asarray(blv, np.float64).reshape(H, L)
    c_h = blv64 @ Wvr64 + np.asarray(bvr, np.float64)[None, :]   # [H, DV]
    const_vec = np.einsum("hv,hvd->d", c_h, Wo64) + np.asarray(bo, np.float64)

    qTh, kTh, vTh = [
        np.asarray(x, f).reshape(B, S, D) for x in (queries, keys, values)
    ]
    Wlk_r = np.asarray(Wlk, f).reshape(D, H, L)
    Wlv_r = np.asarray(Wlv, f).reshape(D, H, L)

    in_maps = []
    xT_cache = {}
    for c in range(N_CORES):
        b = c // BPC
        h0 = (c % BPC) * HPC  # first head of this core's group
        hs = slice(h0, h0 + HPC)
        wq_c = Wqp[:, hs, :].reshape(D, HPC * L)
        wlk_c = Wlk_r[:, hs, :].reshape(D, HPC * L)
        wlv_c = Wlv_r[:, hs, :].reshape(D, HPC * L)
        # W'' pair-stacked: pair p rows = [head 2p (64) | head 2p+1 (64)]
        wop_c = Wop[hs, :, :].reshape(NP, 2 * L, D).transpose(1, 0, 2)
        wop_c = np.ascontiguousarray(wop_c.reshape(128, NP * D).astype(bf))
        bq_c = np.ascontiguousarray(
            bqp_all[hs, :].reshape(NP, 2 * L).T.astype(f)
        )
        if b not in xT_cache:
            def tile_x(x):
                # [S, D] -> [NCH*128, KO*CHUNK] matching the device tiles
                xt = x.T.reshape(KO, 128, NCH, CHUNK)
                return np.ascontiguousarray(
                    xt.transpose(2, 1, 0, 3).reshape(NCH * 128, KO * CHUNK)
                    .astype(bf))
            xT_cache[b] = (tile_x(qTh[b]), tile_x(kTh[b]), tile_x(vTh[b]))
        qTb, kTb, vTb = xT_cache[b]
        in_maps.append({
            "qT": qTb, "kT": kTb, "vT": vTb,
            "wqp": swz(wq_c), "bqp": bq_c,
            "wlk": swz(wlk_c), "wlv": swz(wlv_c),
            "wop": wop_c,
        })
    _prep_in_maps.const_vec = const_vec
    return in_maps


def _assemble(results, bo):
    const_vec = _prep_in_maps.const_vec
    out = np.zeros((B, S, D), np.float64)
    for c, rmap in enumerate(results):
        out[c // BPC] += rmap["outp"].astype(np.float64)
    out += const_vec[None, None, :]
    return out.astype(np.float32)


def kernel(**inputs):
    from concourse.bass_utils import run_bass_kernel_spmd

    nc = _get_nc()
    in_maps = _prep_in_maps(**inputs)
    res = run_bass_kernel_spmd(
        nc, in_maps, core_ids=list(range(N_CORES)), trace=False
    )
    return _assemble(res.results, inputs["bo"])


if __name__ == "__main__":
    nc = build_kernel()
    print("built ok, instructions:", len(nc.inst_map))
